# revision 8
# baseline (speedup 1.0000x reference)
"""DeformableConv2D (B=8, C=F=256, H=W=64, K=3x3) on 8 Trainium2 NeuronCores.

Sharding: data-parallel over batch - each of the 8 cores processes one sample.

Per-core pipeline (v7):
  1. offset/mask 3x3 SAME convs as shifted bf16 matmuls (f32 PSUM), output
     rows [dy(9) | dx(9) | pad | mask(9)@32]; conv split in two halves with
     the pixel-partition PE transposes interleaved into the second half.
  2. Bilinear pipeline in pixel-partition f32: fused floor/frac over the
     joint dy|dx block, mask sigmoid after the transpose, gather indices
     wrapped to the 16-partition dma_gather layout (log-doubling replicate).
  3. 36 corner-product planes q=(2*xc+yc)*9+k kept as PE-transposed
     wrapped-j rows (plrow[36, 4096] bf16).
  4. Main loop over (chunk=1024 px, tap): ONE dma_gather per unit fetches
     all 4 bilinear corners x 256 ch in a single 2KB elem (xg2 row-pair
     layout); this keeps the GPSIMD engine (the bottleneck) at its floor.
  5. Corner multiplies on DVE against planes replicated across partitions
     by selector-matmul PE broadcasts (PSUM -> bf16 via Act copies); one
     merged yc-add; the xc-sum is folded into the GEMM (x2 contraction).
  6. bf16 GEMM into f32 PSUM [128, 1024] x2; output written in wrapped-j
     column order, host reorders.

kernel(**inputs) takes the FULL batch and returns the FULL [8,256,64,64] f32
output.
"""

import dataclasses
from contextlib import ExitStack

import numpy as np

import concourse.bass as bass
import concourse.bacc as bacc
import concourse.tile as tile
from concourse import mybir
from concourse.bass_utils import run_bass_kernel_spmd

H = W = 64
HW = H * W
C = 256
F = 256
K = 9
OC = 41  # conv out rows: 0-8 dy, 9-17 dx, 32-40 mask
PAD = 8
HP = H + 2 * PAD  # 80
WP = W + 2 * PAD  # 80
NROW = HP * WP  # 6400
H1 = H + 2  # 66
W1 = W + 2
HW1 = H1 * W1  # 4356
MARG = 68

FP32 = mybir.dt.float32
I32 = mybir.dt.int32
BF16 = mybir.dt.bfloat16
I16 = mybir.dt.int16
AX = mybir.AluOpType
AF = mybir.ActivationFunctionType

CHUNK = 1024
NCHUNK = HW // CHUNK  # 4
Q = 4 * K  # 36 planes
NCORES = 8
OUT_NATURAL = True


def host_inputs(x, w_offset, w_mask, w_deform):
    """Per-sample layout prep. x: [C,H,W] float32 one sample."""
    import ml_dtypes

    ins = {}
    xp1 = np.zeros((C, H1, W1), ml_dtypes.bfloat16)
    xp1[:, 1:-1, 1:-1] = x
    ins["xpad1"] = np.ascontiguousarray(xp1.reshape(C, HW1))

    # xg2 row (y, x) = [xpad[y, x, :], xpad[y+1, x, :]]  (bf16)
    xp2 = np.zeros((HP + 1, WP, C), ml_dtypes.bfloat16)
    xp2[PAD : PAD + H, PAD : PAD + W, :] = np.transpose(x, (1, 2, 0)).astype(
        ml_dtypes.bfloat16
    )
    xg2 = np.concatenate([xp2[:-1], xp2[1:]], axis=2)  # [HP, WP, 2C]
    ins["xg2"] = np.ascontiguousarray(xg2.reshape(NROW, 2 * C))

    # conv weights, out-channel order [dy(9) | dx(9) | pad | mask(9) at 32]
    wt = np.zeros((3, 3, C, OC), np.float32)
    wo = np.transpose(w_offset, (2, 3, 1, 0))  # [3,3,C,18]
    wt[:, :, :, 0:9] = wo[:, :, :, 0::2]  # dy_k = offset channel 2k
    wt[:, :, :, 9:18] = wo[:, :, :, 1::2]  # dx_k = offset channel 2k+1
    wt[:, :, :, 32:41] = np.transpose(w_mask, (2, 3, 1, 0))
    ins["wconv"] = np.ascontiguousarray(
        wt.reshape(K, 2, 128, OC), dtype=ml_dtypes.bfloat16
    )

    wd = np.transpose(w_deform.reshape(F, C, K), (2, 1, 0))  # [k, c, f]
    ins["wdef"] = np.ascontiguousarray(
        wd.reshape(K, 2, 128, F).astype(ml_dtypes.bfloat16)
    )

    p = np.arange(HW)
    hh = (p // W).astype(np.float32)
    ww = (p % W).astype(np.float32)
    ky = np.repeat(np.arange(3) - 1, 3).astype(np.float32)
    kx = np.tile(np.arange(3) - 1, 3).astype(np.float32)
    basey = (hh[:, None] + ky[None, :]).reshape(32, 128, K).transpose(1, 0, 2)
    basex = (ww[:, None] + kx[None, :]).reshape(32, 128, K).transpose(1, 0, 2)
    ins["basey"] = np.ascontiguousarray(basey, dtype=np.float32)
    ins["basex"] = np.ascontiguousarray(basex, dtype=np.float32)
    ins["ident"] = np.eye(128, dtype=np.float32)
    ins["ones2"] = np.ones((128, 2), np.float32)
    # sel[p, q, :] = (p == q): stationary that broadcasts plrow row q to all
    # 128 PSUM partitions
    sel = np.zeros((Q, Q, 128), ml_dtypes.bfloat16)
    for q in range(Q):
        sel[q, q, :] = 1.0
    ins["sel"] = sel.reshape(Q, Q * 128)
    return ins


def declare_inputs(nc):
    t = {}
    t["xpad1"] = nc.dram_tensor("xpad1", [C, HW1], BF16, kind="ExternalInput")
    t["xg2"] = nc.dram_tensor("xg2", [NROW, 2 * C], BF16, kind="ExternalInput")
    t["wconv"] = nc.dram_tensor("wconv", [K, 2, 128, OC], BF16, kind="ExternalInput")
    t["wdef"] = nc.dram_tensor("wdef", [K, 2, 128, F], BF16, kind="ExternalInput")
    t["basey"] = nc.dram_tensor("basey", [128, 32, K], FP32, kind="ExternalInput")
    t["basex"] = nc.dram_tensor("basex", [128, 32, K], FP32, kind="ExternalInput")
    t["ident"] = nc.dram_tensor("ident", [128, 128], FP32, kind="ExternalInput")
    t["ones2"] = nc.dram_tensor("ones2", [128, 2], FP32, kind="ExternalInput")
    t["sel"] = nc.dram_tensor("sel", [Q, Q * 128], BF16, kind="ExternalInput")
    # columns in wrapped-j order: j = 16*(32a + t) + b <-> pixel 128t + 16a + b
    t["out"] = nc.dram_tensor("out", [F, HW], FP32, kind="ExternalOutput")
    return t


def build(nc, tc, ctx: ExitStack, t):
    keep = ctx.enter_context(tc.tile_pool(name="keep", bufs=1))

    ident = keep.tile([128, 128], FP32)
    ones2 = keep.tile([128, 2], FP32)
    sel = keep.tile([Q, Q * 128], BF16)
    wdef_sb = keep.tile([128, K * 2 * F], BF16)
    widx = keep.tile([128, K, HW // 16], I16)
    plrow = keep.tile([Q, HW], BF16)  # wrapped-j order plane rows

    def load_aux():
        # off the critical path; issued on Act/DVE DMA queues
        nc.scalar.dma_start(ident[:], t["ident"].ap())
        nc.scalar.dma_start(ones2[:], t["ones2"].ap())
        nc.scalar.dma_start(sel[:], t["sel"].ap())
        nc.scalar.dma_start(
            wdef_sb[:].rearrange("p (k c f) -> p k c f", k=K, c=2),
            t["wdef"].ap().rearrange("k c p f -> p k c f"),
        )

    # gather pool + emitter created up front: each chunk's 9 gathers are
    # emitted inside the prologue right after that chunk's stripe chain, so
    # the Pool engine starts gathering long before the prologue finishes.
    gp = ctx.enter_context(tc.tile_pool(name="gth", bufs=4))
    xg_in = dataclasses.replace(
        t["xg2"].ap(), ap=[[2 * C, NROW - 1], [1, 2 * 2 * C]]
    )  # overlapping row pairs, elem = 4 corners x 256ch
    chunks = [(0, 1024), (1024, 1024), (2048, 1024), (3072, 1024)]
    units = [(ci_, k) for ci_ in range(len(chunks)) for k in range(K)]
    gtiles = {}

    def emit_gather(u):
        ci_, k = units[u]
        col0, width = chunks[ci_]
        g = gp.tile([128, 8, width], BF16, tag=f"g{width}", name=f"g{u}")
        nc.gpsimd.dma_gather(
            g[:],
            xg_in,
            widx[:, k, col0 // 16 : (col0 + width) // 16],
            num_idxs=width,
            num_idxs_reg=width,
            elem_size=2 * 2 * C,
            elem_step=2 * C,
            transpose=True,
            single_packet=False,
        )
        gtiles[u] = g

    # ================= prologue =================
    with tc.tile_pool(name="prol", bufs=1) as prol, tc.tile_pool(
        name="stgp", bufs=4
    ) as stgp, tc.tile_pool(
        name="prps", bufs=2, space="PSUM"
    ) as prps, tc.tile_pool(name="trps", bufs=3, space="PSUM") as trps:
        wconv_sb = prol.tile([128, K * 2 * OC], BF16, tag="wconv")
        nc.sync.dma_start(
            wconv_sb[:].rearrange("p (k c o) -> p k c o", k=K, c=2),
            t["wconv"].ap().rearrange("k c p o -> p k c o"),
        )
        xp1 = [
            prol.tile([128, HW1 + 2 * MARG], BF16, tag=f"xp1_{i}", name=f"xp1_{i}")
            for i in range(2)
        ]
        for i in range(2):
            nc.vector.memset(xp1[i][:, 0:MARG], 0.0)
            nc.vector.memset(xp1[i][:, MARG + HW1 :], 0.0)
            nc.sync.dma_start(
                xp1[i][:, MARG : MARG + HW1], t["xpad1"].ap()[bass.ts(i, 128), :]
            )
        load_aux()

        # conv into two half tiles; pixT transposes interleaved with the
        # second half's matmuls so they overlap on all engines.
        # A: rows 0..39 (tcols 0..18); B: rows 40..65
        JSPLIT = 40 * W1  # 2640
        convA = prol.tile([OC, JSPLIT], FP32, tag="convA")
        convB = prol.tile([OC, HW1 - JSPLIT], FP32, tag="convB")
        NCONV = 4 * W1  # 264 (4 rows, 1 PSUM bank)
        wviews = wconv_sb[:].rearrange("p (k c o) -> p k c o", k=K, c=2)
        pixT = prol.tile([128, 32, OC], FP32, tag="pixT")

        def conv_row(h):  # [OC, W1] view of conv output row h
            if (h + 1) * W1 <= JSPLIT:
                return convA[:, h * W1 : (h + 1) * W1]
            return convB[:, h * W1 - JSPLIT : (h + 1) * W1 - JSPLIT]

        def emit_transpose(tcol):
            h0 = 2 * tcol
            stage = stgp.tile([OC, 128], FP32, tag="tr_stage", name=f"st{tcol}")
            for r in range(2):
                nc.vector.tensor_copy(
                    stage[:, 64 * r : 64 * r + 64],
                    conv_row(h0 + 1 + r)[:, 1 : 1 + W],
                )
            ps = trps.tile([128, OC], FP32, tag="tr_ps")
            nc.tensor.transpose(ps[:], stage[:], ident[:OC, :OC])
            if tcol % 2:
                nc.vector.tensor_copy(pixT[:, tcol, :], ps[:])
            else:
                nc.scalar.copy(pixT[:, tcol, :], ps[:])

        def emit_conv_block(j0):
            n = min(NCONV, HW1 - j0)
            ps = prps.tile([OC, NCONV], FP32, tag="conv_ps")
            first = True
            for ci in range(2):
                for k in range(K):
                    off = (k // 3 - 1) * W1 + (k % 3 - 1)
                    nc.tensor.matmul(
                        ps[:, :n],
                        wviews[:, k, ci, :],
                        xp1[ci][:, MARG + j0 + off : MARG + j0 + off + n],
                        start=first,
                        stop=(ci == 1 and k == K - 1),
                    )
                    first = False
            if j0 < JSPLIT:
                nc.scalar.copy(convA[:, j0 : j0 + n], ps[:, :n])
            else:
                nc.scalar.copy(convB[:, j0 - JSPLIT : j0 - JSPLIT + n], ps[:, :n])

        # ---- per-stripe coefficient/index pipeline ----
        # stripe s covers tcols [8s, 8s+8) = pixels [1024s, 1024s+1024),
        # which is exactly main-loop chunk s. Ops are sliced per stripe so
        # chunk s's indices/planes complete as soon as its conv rows are
        # transposed, letting the gathers start long before the full conv.
        def pt2(tag):
            return prol.tile([128, 32, 2 * K], FP32, tag=tag, name=tag)

        typ = pt2("typ")
        fyx = pt2("fyx")
        wyx = pt2("wyx")
        cr = pt2("cr")
        mwy0 = prol.tile([128, 32, K], FP32, tag="mwy0", name="mwy0")
        mwy1 = prol.tile([128, 32, K], FP32, tag="mwy1", name="mwy1")
        iy = prol.tile([128, 32, 2 * K], I32, tag="iy")
        base2 = prol.tile([128, 32, 2 * K], FP32, tag="base2")
        nc.sync.dma_start(base2[:, :, 0:9], t["basey"].ap())
        nc.sync.dma_start(base2[:, :, 9:18], t["basex"].ap())
        CONST = PAD * WP + PAD
        idxt = prol.tile([128, K, 32], FP32, tag="idxt")
        idxi = prol.tile([128, K, 32], I16, tag="idxi")
        coefq = prol.tile([128, Q, 32], FP32, tag="coefq")
        wv = widx[:].rearrange("p q (t a) -> p q t a", a=8)

        def emit_stripe(s):
            ts = slice(8 * s, 8 * s + 8)
            nc.scalar.activation(
                pixT[:, ts, 32:41], pixT[:, ts, 32:41], AF.Sigmoid
            )
            # fpos = floor(dv + base), robust to trunc-or-round casts
            nc.vector.tensor_add(typ[:, ts], pixT[:, ts, 0:18], base2[:, ts])
            nc.vector.tensor_copy(iy[:, ts], typ[:, ts])
            nc.vector.tensor_copy(fyx[:, ts], iy[:, ts])
            nc.vector.tensor_tensor(cr[:, ts], fyx[:, ts], typ[:, ts], AX.is_gt)
            nc.vector.tensor_sub(fyx[:, ts], fyx[:, ts], cr[:, ts])
            nc.vector.tensor_sub(wyx[:, ts], typ[:, ts], fyx[:, ts])
            fy = fyx[:, ts, 0:9]
            fx = fyx[:, ts, 9:18]
            wy = wyx[:, ts, 0:9]
            wx = wyx[:, ts, 9:18]
            mv = pixT[:, ts, 32:41]
            # gather indices
            iv = idxt[:, :, ts].rearrange("p q t -> p t q")
            nc.vector.scalar_tensor_tensor(iv, fy, float(WP), fx, AX.mult, AX.add)
            nc.vector.tensor_scalar_add(iv, iv, float(CONST))
            nc.vector.tensor_scalar(
                idxt[:, :, ts], idxt[:, :, ts], 0.0, float(NROW - 2), AX.max, AX.min
            )
            nc.vector.tensor_copy(idxi[:, :, ts], idxt[:, :, ts])
            # wrap (identity mapping) + replicate for this stripe's columns
            for a in range(8):
                eng = (nc.sync, nc.scalar)[a % 2]
                eng.dma_start(
                    widx[0:16, :, 64 * s + 8 * a : 64 * s + 8 * a + 8],
                    idxi[16 * a : 16 * a + 16, :, ts],
                )
            for st in range(3):
                w = 16 << st
                eng = (nc.sync, nc.scalar)[s % 2]
                eng.dma_start(
                    widx[w : 2 * w, :, 64 * s : 64 * s + 64],
                    widx[0:w, :, 64 * s : 64 * s + 64],
                )
            # corner-product planes
            nc.vector.tensor_mul(mwy1[:, ts], mv, wy)
            nc.vector.tensor_sub(mwy0[:, ts], mv, mwy1[:, ts])
            cv = coefq[:, :, ts].rearrange("p q t -> p t q")
            nc.vector.tensor_mul(cv[:, :, 18:27], mwy0[:, ts], wx)
            nc.vector.tensor_sub(cv[:, :, 0:9], mwy0[:, ts], cv[:, :, 18:27])
            nc.vector.tensor_mul(cv[:, :, 27:36], mwy1[:, ts], wx)
            nc.vector.tensor_sub(cv[:, :, 9:18], mwy1[:, ts], cv[:, :, 27:36])
            for tcol in range(8 * s, 8 * s + 8):
                tm = tcol - 8 * s
                stage2 = stgp.tile([128, Q], FP32, tag="tr2_stage", name=f"s2{tcol}")
                nc.vector.tensor_copy(stage2[:], coefq[:, :, tcol])
                ps = trps.tile([Q, 128], FP32, tag="tr2_ps")
                nc.tensor.transpose(ps[:], stage2[:], ident[:, :])
                dstr = plrow[:, 0:128]
                dstr = dataclasses.replace(
                    dstr,
                    ap=[list(dstr.ap[0]), [128, 8], [1, 16]],
                    offset=dstr.offset + 1024 * s + 16 * tm,
                )
                eng = nc.vector.tensor_copy if tcol % 2 else nc.scalar.copy
                eng(dstr, ps[:].rearrange("q (a b) -> q a b", a=8))


        # interleave pixT transposes (and each stripe's index/plane chain)
        # into the conv as rows become ready (tcol T reads rows 2T+1, 2T+2)
        pend = list(range(32))

        def drain_ready(rows_done):
            while pend and 2 * pend[0] + 2 < rows_done:
                tcol = pend.pop(0)
                emit_transpose(tcol)
                if tcol % 8 == 7:
                    s = tcol // 8
                    emit_stripe(s)
                    for u in range(9 * s, 9 * s + 9):
                        emit_gather(u)

        for j0 in range(0, HW1, NCONV):
            emit_conv_block(j0)
            drain_ready(min(j0 + NCONV, HW1) // W1)
        for tcol in list(pend):
            pend.pop(0)
            emit_transpose(tcol)
            if tcol % 8 == 7:
                s = tcol // 8
                emit_stripe(s)
                for u in range(9 * s, 9 * s + 9):
                    emit_gather(u)


    # ================= main loop (computes; gathers emitted above) ========
    ap_pool = ctx.enter_context(tc.tile_pool(name="amul", bufs=3))
    pr_pool = ctx.enter_context(tc.tile_pool(name="prep", bufs=3))
    sp = ctx.enter_context(tc.tile_pool(name="sums", bufs=2))
    op = ctx.enter_context(tc.tile_pool(name="outp", bufs=2))
    gps = ctx.enter_context(tc.tile_pool(name="gemm_ps", bufs=1, space="PSUM"))
    bps = ctx.enter_context(tc.tile_pool(name="brd_ps", bufs=2, space="PSUM"))

    wdef_v = wdef_sb[:].rearrange("p (k c f) -> p k c f", k=K, c=2)

    ps_out = {}
    for v in range(len(units)):
        ci_, k = units[v]
        col0, width = chunks[ci_]
        ch = ci_
        if k == 0:
            ps_out[ch] = [
                gps.tile([128, CHUNK], FP32, tag=f"ops{m}", name=f"ops{ch}_{m}")
                for m in range(2)
            ]
        g = gtiles.pop(v)
        a = ap_pool.tile([128, 2, 2, 2, CHUNK], BF16, tag="am", name=f"am{v}")
        for xc in range(2):
            prep = pr_pool.tile(
                [128, 2, CHUNK], BF16, tag="prep", name=f"pr{v}_{xc}"
            )
            for yc in range(2):
                q = (2 * xc + yc) * K + k
                brd = bps.tile([128, CHUNK], FP32, tag="brd", name=f"brd{v}_{xc}{yc}")
                for n0 in range(0, width, 512):
                    nc.tensor.matmul(
                        brd[:, n0 : n0 + 512],
                        sel[:, 128 * q : 128 * q + 128],
                        plrow[:, col0 + n0 : col0 + n0 + 512],
                        start=True,
                        stop=True,
                    )
                nc.scalar.copy(prep[:, yc, :width], brd[:, :width])
            pr_ap = prep[:]
            pr_b = dataclasses.replace(
                pr_ap,
                ap=[list(pr_ap.ap[0]), [CHUNK, 2], [0, 2], [1, width]],
            )
            nc.vector.tensor_tensor(
                a[:, xc, :, :, :width],
                g[:, 4 * xc : 4 * xc + 4, :width].rearrange(
                    "p (y c) j -> p y c j", y=2
                ),
                pr_b,
                AX.mult,
            )
        # single merged add over yc: s[xc, ci, j] = sum_yc am[xc, yc, ci, j]
        s01 = sp.tile([128, 2, 2, CHUNK], BF16, tag="s01", name=f"s01_{v}")
        nc.vector.tensor_add(
            s01[:, :, :, :width], a[:, :, 0, :, :width], a[:, :, 1, :, :width]
        )

        first = k == 0
        last = k == K - 1
        for m in range(2):
            for si in range(2):
                for ci in range(2):
                    for n0 in range(0, width, 512):
                        nc.tensor.matmul(
                            ps_out[ch][m][:, n0 : n0 + 512],
                            wdef_v[:, k, ci, bass.ts(m, 128)],
                            s01[:, si, ci, n0 : n0 + 512],
                            start=(first and si == 0 and ci == 0),
                            stop=(last and si == 1 and ci == 1),
                        )
        if k == K - 1:
            for m in range(2):
                ot = op.tile([128, CHUNK], FP32, tag="ot", name=f"ot{ch}_{m}")
                nc.scalar.copy(ot[:, :width], ps_out[ch][m][:, :width])
                nc.sync.dma_start(
                    t["out"].ap()[bass.ts(m, 128), col0 : col0 + width],
                    ot[:, :width],
                )
            ps_out.pop(ch)


_CACHE = {}


def _get_nc():
    if "nc" not in _CACHE:
        nc = bacc.Bacc("TRN2", target_bir_lowering=False, num_devices=NCORES)
        t = declare_inputs(nc)
        with tile.TileContext(nc) as tc:
            with ExitStack() as ctx:
                build(nc, tc, ctx, t)
        nc.finalize()
        _CACHE["nc"] = nc
    return _CACHE["nc"]


def kernel(x, w_offset, w_mask, w_deform):
    """Full-batch deformable conv. x: [8,256,64,64] f32 -> [8,256,64,64] f32."""
    x = np.asarray(x, dtype=np.float32)
    w_offset = np.asarray(w_offset, dtype=np.float32)
    w_mask = np.asarray(w_mask, dtype=np.float32)
    w_deform = np.asarray(w_deform, dtype=np.float32)
    B = x.shape[0]
    assert B == NCORES
    nc = _get_nc()
    in_maps = [host_inputs(x[b], w_offset, w_mask, w_deform) for b in range(B)]
    res = run_bass_kernel_spmd(nc, in_maps, list(range(NCORES)))
    out = np.empty((B, F, H, W), np.float32)
    for b in range(B):
        o = res.results[b]["out"].reshape(F, 4, 8, 8, 16)  # (s, a, tm, b)
        out[b] = o.transpose(0, 1, 3, 2, 4).reshape(F, H, W)
    return out


# revision 9
# speedup vs baseline: 1.0076x; 1.0076x over previous
"""DeformableConv2D (B=8, C=F=256, H=W=64, K=3x3) on 8 Trainium2 NeuronCores.

Sharding: data-parallel over batch - each of the 8 cores processes one sample.

Per-core pipeline (v8):
  1. offset/mask 3x3 SAME convs as shifted bf16 matmuls (f32 PSUM), output
     rows [dy(9) | dx(9) | pad | mask(9)@32], emitted in 4-row blocks.
  2. The image is processed in four 16-row stripes (= main-loop chunks):
     as each stripe's conv rows finish, its pixel-partition PE transposes,
     fused floor/frac pipeline, gather-index wrap (+replicate to the
     16-partition dma_gather layout), corner-product planes, plane-row
     transposes, AND its 9 dma_gathers are emitted - so the GPSIMD engine
     (the bottleneck at ~246us of gathers) starts ~25us into the prologue.
  3. Main loop over (chunk=1024 px, tap): ONE dma_gather per unit fetched
     all 4 bilinear corners x 256 ch in a single 2KB elem (xg2 row-pair
     layout, single_packet=False).
  4. Corner multiplies on DVE against planes replicated across partitions
     by selector-matmul PE broadcasts (PSUM -> bf16 via Act copies); one
     merged yc-add; the xc-sum is folded into the GEMM (x2 contraction).
  5. bf16 GEMM into f32 PSUM [128, 1024] x2; output columns are pixels in
     (stripe, a, tm, b) order; the host applies the inverse permute.

kernel(**inputs) takes the FULL batch and returns the FULL [8,256,64,64] f32
output.
"""

import dataclasses
from contextlib import ExitStack

import numpy as np

import concourse.bass as bass
import concourse.bacc as bacc
import concourse.tile as tile
from concourse import mybir
from concourse.bass_utils import run_bass_kernel_spmd

H = W = 64
HW = H * W
C = 256
F = 256
K = 9
OC = 41  # conv out rows: 0-8 dy, 9-17 dx, 32-40 mask
PAD = 8
HP = H + 2 * PAD  # 80
WP = W + 2 * PAD  # 80
NROW = HP * WP  # 6400
H1 = H + 2  # 66
W1 = W + 2
HW1 = H1 * W1  # 4356
MARG = 68

FP32 = mybir.dt.float32
I32 = mybir.dt.int32
BF16 = mybir.dt.bfloat16
I16 = mybir.dt.int16
AX = mybir.AluOpType
AF = mybir.ActivationFunctionType

CHUNK = 1024
NCHUNK = HW // CHUNK  # 4
Q = 4 * K  # 36 planes
NCORES = 8
OUT_NATURAL = True


def host_inputs(x, w_offset, w_mask, w_deform):
    """Per-sample layout prep. x: [C,H,W] float32 one sample."""
    import ml_dtypes

    ins = {}
    xp1 = np.zeros((C, H1, W1), ml_dtypes.bfloat16)
    xp1[:, 1:-1, 1:-1] = x
    ins["xpad1"] = np.ascontiguousarray(xp1.reshape(C, HW1))

    # xg2 row (y, x) = [xpad[y, x, :], xpad[y+1, x, :]]  (bf16)
    xp2 = np.zeros((HP + 1, WP, C), ml_dtypes.bfloat16)
    xp2[PAD : PAD + H, PAD : PAD + W, :] = np.transpose(x, (1, 2, 0)).astype(
        ml_dtypes.bfloat16
    )
    xg2 = np.concatenate([xp2[:-1], xp2[1:]], axis=2)  # [HP, WP, 2C]
    ins["xg2"] = np.ascontiguousarray(xg2.reshape(NROW, 2 * C))

    # conv weights, out-channel order [dy(9) | dx(9) | pad | mask(9) at 32]
    wt = np.zeros((3, 3, C, OC), np.float32)
    wo = np.transpose(w_offset, (2, 3, 1, 0))  # [3,3,C,18]
    wt[:, :, :, 0:9] = wo[:, :, :, 0::2]  # dy_k = offset channel 2k
    wt[:, :, :, 9:18] = wo[:, :, :, 1::2]  # dx_k = offset channel 2k+1
    wt[:, :, :, 32:41] = np.transpose(w_mask, (2, 3, 1, 0))
    ins["wconv"] = np.ascontiguousarray(
        wt.reshape(K, 2, 128, OC), dtype=ml_dtypes.bfloat16
    )

    wd = np.transpose(w_deform.reshape(F, C, K), (2, 1, 0))  # [k, c, f]
    ins["wdef"] = np.ascontiguousarray(
        wd.reshape(K, 2, 128, F).astype(ml_dtypes.bfloat16)
    )

    p = np.arange(HW)
    hh = (p // W).astype(np.float32)
    ww = (p % W).astype(np.float32)
    ky = np.repeat(np.arange(3) - 1, 3).astype(np.float32)
    kx = np.tile(np.arange(3) - 1, 3).astype(np.float32)
    basey = (hh[:, None] + ky[None, :]).reshape(32, 128, K).transpose(1, 0, 2)
    basex = (ww[:, None] + kx[None, :]).reshape(32, 128, K).transpose(1, 0, 2)
    ins["basey"] = np.ascontiguousarray(basey, dtype=np.float32)
    ins["basex"] = np.ascontiguousarray(basex, dtype=np.float32)
    ins["ident"] = np.eye(128, dtype=np.float32)
    ins["ones2"] = np.ones((128, 2), np.float32)
    # sel[p, q, :] = (p == q): stationary that broadcasts plrow row q to all
    # 128 PSUM partitions
    sel = np.zeros((Q, Q, 128), ml_dtypes.bfloat16)
    for q in range(Q):
        sel[q, q, :] = 1.0
    ins["sel"] = sel.reshape(Q, Q * 128)
    return ins


def declare_inputs(nc):
    t = {}
    t["xpad1"] = nc.dram_tensor("xpad1", [C, HW1], BF16, kind="ExternalInput")
    t["xg2"] = nc.dram_tensor("xg2", [NROW, 2 * C], BF16, kind="ExternalInput")
    t["wconv"] = nc.dram_tensor("wconv", [K, 2, 128, OC], BF16, kind="ExternalInput")
    t["wdef"] = nc.dram_tensor("wdef", [K, 2, 128, F], BF16, kind="ExternalInput")
    t["basey"] = nc.dram_tensor("basey", [128, 32, K], FP32, kind="ExternalInput")
    t["basex"] = nc.dram_tensor("basex", [128, 32, K], FP32, kind="ExternalInput")
    t["ident"] = nc.dram_tensor("ident", [128, 128], FP32, kind="ExternalInput")
    t["ones2"] = nc.dram_tensor("ones2", [128, 2], FP32, kind="ExternalInput")
    t["sel"] = nc.dram_tensor("sel", [Q, Q * 128], BF16, kind="ExternalInput")
    # columns in wrapped-j order: j = 16*(32a + t) + b <-> pixel 128t + 16a + b
    t["out"] = nc.dram_tensor("out", [F, HW], FP32, kind="ExternalOutput")
    return t


def build(nc, tc, ctx: ExitStack, t):
    keep = ctx.enter_context(tc.tile_pool(name="keep", bufs=1))

    ident = keep.tile([128, 128], FP32)
    ones2 = keep.tile([128, 2], FP32)
    sel = keep.tile([Q, Q * 128], BF16)
    wdef_sb = keep.tile([128, K * 2 * F], BF16)
    widx = keep.tile([128, K, HW // 16], I16)
    plrow = keep.tile([Q, HW], BF16)  # wrapped-j order plane rows

    def load_aux():
        # off the critical path; issued on Act/DVE DMA queues
        nc.scalar.dma_start(ident[:], t["ident"].ap())
        nc.scalar.dma_start(ones2[:], t["ones2"].ap())
        nc.scalar.dma_start(sel[:], t["sel"].ap())
        nc.scalar.dma_start(
            wdef_sb[:].rearrange("p (k c f) -> p k c f", k=K, c=2),
            t["wdef"].ap().rearrange("k c p f -> p k c f"),
        )

    # gather pool + emitter created up front: each chunk's 9 gathers are
    # emitted inside the prologue right after that chunk's stripe chain, so
    # the Pool engine starts gathering long before the prologue finishes.
    gp = ctx.enter_context(tc.tile_pool(name="gth", bufs=4))
    xg_in = dataclasses.replace(
        t["xg2"].ap(), ap=[[2 * C, NROW - 1], [1, 2 * 2 * C]]
    )  # overlapping row pairs, elem = 4 corners x 256ch
    chunks = [(0, 1024), (1024, 1024), (2048, 1024), (3072, 1024)]
    units = [(ci_, k) for ci_ in range(len(chunks)) for k in range(K)]
    gtiles = {}

    def emit_gather(u):
        ci_, k = units[u]
        col0, width = chunks[ci_]
        g = gp.tile([128, 8, width], BF16, tag=f"g{width}", name=f"g{u}")
        nc.gpsimd.dma_gather(
            g[:],
            xg_in,
            widx[:, k, col0 // 16 : (col0 + width) // 16],
            num_idxs=width,
            num_idxs_reg=width,
            elem_size=2 * 2 * C,
            elem_step=2 * C,
            transpose=True,
            single_packet=False,
        )
        gtiles[u] = g

    # ================= prologue =================
    with tc.tile_pool(name="prol", bufs=1) as prol, tc.tile_pool(
        name="stgp", bufs=4
    ) as stgp, tc.tile_pool(
        name="prps", bufs=2, space="PSUM"
    ) as prps, tc.tile_pool(name="trps", bufs=3, space="PSUM") as trps:
        wconv_sb = prol.tile([128, K * 2 * OC], BF16, tag="wconv")
        nc.sync.dma_start(
            wconv_sb[:].rearrange("p (k c o) -> p k c o", k=K, c=2),
            t["wconv"].ap().rearrange("k c p o -> p k c o"),
        )
        xp1 = [
            prol.tile([128, HW1 + 2 * MARG], BF16, tag=f"xp1_{i}", name=f"xp1_{i}")
            for i in range(2)
        ]
        for i in range(2):
            nc.vector.memset(xp1[i][:, 0:MARG], 0.0)
            nc.vector.memset(xp1[i][:, MARG + HW1 :], 0.0)
            nc.sync.dma_start(
                xp1[i][:, MARG : MARG + HW1], t["xpad1"].ap()[bass.ts(i, 128), :]
            )
        load_aux()

        # conv into two half tiles; pixT transposes interleaved with the
        # second half's matmuls so they overlap on all engines.
        # A: rows 0..39 (tcols 0..18); B: rows 40..65
        JSPLIT = 40 * W1  # 2640
        convA = prol.tile([OC, JSPLIT], FP32, tag="convA")
        convB = prol.tile([OC, HW1 - JSPLIT], FP32, tag="convB")
        NCONV = 4 * W1  # 264 (4 rows, 1 PSUM bank)
        wviews = wconv_sb[:].rearrange("p (k c o) -> p k c o", k=K, c=2)
        pixT = prol.tile([128, 32, OC], FP32, tag="pixT")

        def conv_row(h):  # [OC, W1] view of conv output row h
            if (h + 1) * W1 <= JSPLIT:
                return convA[:, h * W1 : (h + 1) * W1]
            return convB[:, h * W1 - JSPLIT : (h + 1) * W1 - JSPLIT]

        def emit_transpose(tcol):
            h0 = 2 * tcol
            stage = stgp.tile([OC, 128], FP32, tag="tr_stage", name=f"st{tcol}")
            for r in range(2):
                nc.vector.tensor_copy(
                    stage[:, 64 * r : 64 * r + 64],
                    conv_row(h0 + 1 + r)[:, 1 : 1 + W],
                )
            ps = trps.tile([128, OC], FP32, tag="tr_ps")
            nc.tensor.transpose(ps[:], stage[:], ident[:OC, :OC])
            if tcol % 2:
                nc.vector.tensor_copy(pixT[:, tcol, :], ps[:])
            else:
                nc.scalar.copy(pixT[:, tcol, :], ps[:])

        def emit_conv_block(j0):
            n = min(NCONV, HW1 - j0)
            ps = prps.tile([OC, NCONV], FP32, tag="conv_ps")
            first = True
            for ci in range(2):
                for k in range(K):
                    off = (k // 3 - 1) * W1 + (k % 3 - 1)
                    nc.tensor.matmul(
                        ps[:, :n],
                        wviews[:, k, ci, :],
                        xp1[ci][:, MARG + j0 + off : MARG + j0 + off + n],
                        start=first,
                        stop=(ci == 1 and k == K - 1),
                    )
                    first = False
            if j0 < JSPLIT:
                nc.scalar.copy(convA[:, j0 : j0 + n], ps[:, :n])
            else:
                nc.scalar.copy(convB[:, j0 - JSPLIT : j0 - JSPLIT + n], ps[:, :n])

        # ---- per-stripe coefficient/index pipeline ----
        # stripe s covers tcols [8s, 8s+8) = pixels [1024s, 1024s+1024),
        # which is exactly main-loop chunk s. Ops are sliced per stripe so
        # chunk s's indices/planes complete as soon as its conv rows are
        # transposed, letting the gathers start long before the full conv.
        def pt2(tag):
            return prol.tile([128, 32, 2 * K], FP32, tag=tag, name=tag)

        typ = pt2("typ")
        fyx = pt2("fyx")
        wyx = pt2("wyx")
        cr = pt2("cr")
        mwy0 = prol.tile([128, 32, K], FP32, tag="mwy0", name="mwy0")
        mwy1 = prol.tile([128, 32, K], FP32, tag="mwy1", name="mwy1")
        iy = prol.tile([128, 32, 2 * K], I32, tag="iy")
        base2 = prol.tile([128, 32, 2 * K], FP32, tag="base2")
        nc.sync.dma_start(base2[:, :, 0:9], t["basey"].ap())
        nc.sync.dma_start(base2[:, :, 9:18], t["basex"].ap())
        CONST = PAD * WP + PAD
        idxt = prol.tile([128, K, 32], FP32, tag="idxt")
        idxi = prol.tile([128, K, 32], I16, tag="idxi")
        coefq = prol.tile([128, Q, 32], FP32, tag="coefq")
        wv = widx[:].rearrange("p q (t a) -> p q t a", a=8)

        def emit_stripe(s):
            ts = slice(8 * s, 8 * s + 8)
            nc.scalar.activation(
                pixT[:, ts, 32:41], pixT[:, ts, 32:41], AF.Sigmoid
            )
            # fpos = floor(dv + base), robust to trunc-or-round casts
            nc.vector.tensor_add(typ[:, ts], pixT[:, ts, 0:18], base2[:, ts])
            nc.vector.tensor_copy(iy[:, ts], typ[:, ts])
            nc.vector.tensor_copy(fyx[:, ts], iy[:, ts])
            nc.vector.tensor_tensor(cr[:, ts], fyx[:, ts], typ[:, ts], AX.is_gt)
            nc.vector.tensor_sub(fyx[:, ts], fyx[:, ts], cr[:, ts])
            nc.vector.tensor_sub(wyx[:, ts], typ[:, ts], fyx[:, ts])
            fy = fyx[:, ts, 0:9]
            fx = fyx[:, ts, 9:18]
            wy = wyx[:, ts, 0:9]
            wx = wyx[:, ts, 9:18]
            mv = pixT[:, ts, 32:41]
            # gather indices
            iv = idxt[:, :, ts].rearrange("p q t -> p t q")
            nc.vector.scalar_tensor_tensor(iv, fy, float(WP), fx, AX.mult, AX.add)
            nc.vector.tensor_scalar_add(iv, iv, float(CONST))
            nc.vector.tensor_scalar(
                idxt[:, :, ts], idxt[:, :, ts], 0.0, float(NROW - 2), AX.max, AX.min
            )
            nc.vector.tensor_copy(idxi[:, :, ts], idxt[:, :, ts])
            # wrap (identity mapping) + replicate for this stripe's columns
            for a in range(8):
                eng = (nc.sync, nc.scalar)[a % 2]
                eng.dma_start(
                    widx[0:16, :, 64 * s + 8 * a : 64 * s + 8 * a + 8],
                    idxi[16 * a : 16 * a + 16, :, ts],
                )
            for st in range(3):
                w = 16 << st
                eng = (nc.sync, nc.scalar)[s % 2]
                eng.dma_start(
                    widx[w : 2 * w, :, 64 * s : 64 * s + 64],
                    widx[0:w, :, 64 * s : 64 * s + 64],
                )
            # corner-product planes
            nc.vector.tensor_mul(mwy1[:, ts], mv, wy)
            nc.vector.tensor_sub(mwy0[:, ts], mv, mwy1[:, ts])
            cv = coefq[:, :, ts].rearrange("p q t -> p t q")
            nc.vector.tensor_mul(cv[:, :, 18:27], mwy0[:, ts], wx)
            nc.vector.tensor_sub(cv[:, :, 0:9], mwy0[:, ts], cv[:, :, 18:27])
            nc.vector.tensor_mul(cv[:, :, 27:36], mwy1[:, ts], wx)
            nc.vector.tensor_sub(cv[:, :, 9:18], mwy1[:, ts], cv[:, :, 27:36])
            for tcol in range(8 * s, 8 * s + 8):
                tm = tcol - 8 * s
                stage2 = stgp.tile([128, Q], FP32, tag="tr2_stage", name=f"s2{tcol}")
                nc.vector.tensor_copy(stage2[:], coefq[:, :, tcol])
                ps = trps.tile([Q, 128], FP32, tag="tr2_ps")
                nc.tensor.transpose(ps[:], stage2[:], ident[:, :])
                dstr = plrow[:, 0:128]
                dstr = dataclasses.replace(
                    dstr,
                    ap=[list(dstr.ap[0]), [128, 8], [1, 16]],
                    offset=dstr.offset + 1024 * s + 16 * tm,
                )
                eng = nc.vector.tensor_copy if tcol % 2 else nc.scalar.copy
                eng(dstr, ps[:].rearrange("q (a b) -> q a b", a=8))


        # interleave pixT transposes (and each stripe's index/plane chain)
        # into the conv as rows become ready (tcol T reads rows 2T+1, 2T+2)
        pend = list(range(32))

        def drain_ready(rows_done):
            while pend and 2 * pend[0] + 2 < rows_done:
                tcol = pend.pop(0)
                emit_transpose(tcol)
                if tcol % 8 == 7:
                    s = tcol // 8
                    emit_stripe(s)
                    for u in range(9 * s, 9 * s + 9):
                        emit_gather(u)

        for j0 in range(0, HW1, NCONV):
            emit_conv_block(j0)
            drain_ready(min(j0 + NCONV, HW1) // W1)
        for tcol in list(pend):
            pend.pop(0)
            emit_transpose(tcol)
            if tcol % 8 == 7:
                s = tcol // 8
                emit_stripe(s)
                for u in range(9 * s, 9 * s + 9):
                    emit_gather(u)


    # ================= main loop (computes; gathers emitted above) ========
    ap_pool = ctx.enter_context(tc.tile_pool(name="amul", bufs=3))
    pr_pool = ctx.enter_context(tc.tile_pool(name="prep", bufs=3))
    sp = ctx.enter_context(tc.tile_pool(name="sums", bufs=2))
    op = ctx.enter_context(tc.tile_pool(name="outp", bufs=2))
    gps = ctx.enter_context(tc.tile_pool(name="gemm_ps", bufs=1, space="PSUM"))
    bps = ctx.enter_context(tc.tile_pool(name="brd_ps", bufs=2, space="PSUM"))

    wdef_v = wdef_sb[:].rearrange("p (k c f) -> p k c f", k=K, c=2)

    ps_out = {}
    for v in range(len(units)):
        ci_, k = units[v]
        col0, width = chunks[ci_]
        ch = ci_
        if k == 0:
            ps_out[ch] = [
                gps.tile([128, CHUNK], FP32, tag=f"ops{m}", name=f"ops{ch}_{m}")
                for m in range(2)
            ]
        g = gtiles.pop(v)
        a = ap_pool.tile([128, 2, 2, 2, CHUNK], BF16, tag="am", name=f"am{v}")
        for xc in range(2):
            prep = pr_pool.tile(
                [128, 2, CHUNK], BF16, tag="prep", name=f"pr{v}_{xc}"
            )
            for yc in range(2):
                q = (2 * xc + yc) * K + k
                brd = bps.tile([128, CHUNK], FP32, tag="brd", name=f"brd{v}_{xc}{yc}")
                for n0 in range(0, width, 512):
                    nc.tensor.matmul(
                        brd[:, n0 : n0 + 512],
                        sel[:, 128 * q : 128 * q + 128],
                        plrow[:, col0 + n0 : col0 + n0 + 512],
                        start=True,
                        stop=True,
                    )
                nc.scalar.copy(prep[:, yc, :width], brd[:, :width])
            pr_ap = prep[:]
            pr_b = dataclasses.replace(
                pr_ap,
                ap=[list(pr_ap.ap[0]), [CHUNK, 2], [0, 2], [1, width]],
            )
            nc.vector.tensor_tensor(
                a[:, xc, :, :, :width],
                g[:, 4 * xc : 4 * xc + 4, :width].rearrange(
                    "p (y c) j -> p y c j", y=2
                ),
                pr_b,
                AX.mult,
            )
        # single merged add over yc: s[xc, ci, j] = sum_yc am[xc, yc, ci, j]
        s01 = sp.tile([128, 2, 2, CHUNK], BF16, tag="s01", name=f"s01_{v}")
        nc.vector.tensor_add(
            s01[:, :, :, :width], a[:, :, 0, :, :width], a[:, :, 1, :, :width]
        )

        first = k == 0
        last = k == K - 1
        for m in range(2):
            for si in range(2):
                for ci in range(2):
                    for n0 in range(0, width, 512):
                        nc.tensor.matmul(
                            ps_out[ch][m][:, n0 : n0 + 512],
                            wdef_v[:, k, ci, bass.ts(m, 128)],
                            s01[:, si, ci, n0 : n0 + 512],
                            start=(first and si == 0 and ci == 0),
                            stop=(last and si == 1 and ci == 1),
                        )
        if k == K - 1:
            for m in range(2):
                ot = op.tile([128, CHUNK], FP32, tag="ot", name=f"ot{ch}_{m}")
                nc.scalar.copy(ot[:, :width], ps_out[ch][m][:, :width])
                nc.sync.dma_start(
                    t["out"].ap()[bass.ts(m, 128), col0 : col0 + width],
                    ot[:, :width],
                )
            ps_out.pop(ch)


_CACHE = {}


def _get_nc():
    if "nc" not in _CACHE:
        nc = bacc.Bacc("TRN2", target_bir_lowering=False, num_devices=NCORES)
        t = declare_inputs(nc)
        with tile.TileContext(nc) as tc:
            with ExitStack() as ctx:
                build(nc, tc, ctx, t)
        nc.finalize()
        _CACHE["nc"] = nc
    return _CACHE["nc"]


def kernel(x, w_offset, w_mask, w_deform):
    """Full-batch deformable conv. x: [8,256,64,64] f32 -> [8,256,64,64] f32."""
    x = np.asarray(x, dtype=np.float32)
    w_offset = np.asarray(w_offset, dtype=np.float32)
    w_mask = np.asarray(w_mask, dtype=np.float32)
    w_deform = np.asarray(w_deform, dtype=np.float32)
    B = x.shape[0]
    assert B == NCORES
    nc = _get_nc()
    in_maps = [host_inputs(x[b], w_offset, w_mask, w_deform) for b in range(B)]
    res = run_bass_kernel_spmd(nc, in_maps, list(range(NCORES)))
    out = np.empty((B, F, H, W), np.float32)
    for b in range(B):
        o = res.results[b]["out"].reshape(F, 4, 8, 8, 16)  # (s, a, tm, b)
        out[b] = o.transpose(0, 1, 3, 2, 4).reshape(F, H, W)
    return out
